# revision 1
# baseline (speedup 1.0000x reference)
"""Trainium2 Bass kernel for nn_CascadeTransformerMM (4-layer ternary-GLU cascade).

v8: deep software pipeline, PE-gap-free steady state:
  - q-pass (RMS-scale + act-quant + transpose) runs 2 tiles ahead; row stats
    for layer l+1 fold into layer l's tile tails, computed in two halves so
    the next layer's first q-passes never wait on tile 7.
  - down-projection lags two tiles behind up-projection on the PE queue; the
    gq magic-round (2x [128,4096] scalar ops) is chunked into [128,512]
    pieces interleaved between the NEXT tile's silu ops so the scalar queue
    never head-of-line-blocks a silu (which gates PE via PSUM rotation).
  - weights stream fp32 (fp16 flips ternary signs near threshold; the
    cascade amplifies that to ~3e-2); ternarize = fp32 -> int16 -> fp8 on
    DVE in [128,1024] units, 2 units interleaved per fg-slot of the
    previous layer's tiles 2..7; staging writes ride gpsimd with lag-1.
  - per-matrix |W|max scales are computed host-side (12 scalars).

Math (per layer, per token row):
  h   = rms_scale * x * rsqrt(mean(x^2) + 1e-6)
  s   = clip(127/(max|h| + 1e-5), 1e-3, 1e3);  q = round(s*h)
  Wt  = clip(round(W * 127/(max|W| + 1e-5)), -1, 1)      (ternary {-1,0,1})
  u   = (q @ Wg_t)/s ; v = (q @ Wu_t)/s ; g = silu(u)*v
  s2  = clip(127/(max|g| + 1e-5), 1e-3, 1e3); gq = round(s2*g)
  x  += (gq @ Wd_t)/s2

Distribution: data-parallel over batch (8 batches -> 8 cores), weights
replicated per core.
"""

import os
import sys

for _p in ("/opt/trn_rl_repo", "/root/.axon_site/_ro/trn_rl_repo"):
    if os.path.isdir(_p) and _p not in sys.path:
        sys.path.insert(0, _p)

import numpy as np
from contextlib import ExitStack

import concourse.bass as bass
import concourse.mybir as mybir
import concourse.tile as tile
from concourse.bass_utils import run_bass_kernel_spmd

dt = mybir.dt
AF = mybir.ActivationFunctionType
ALU = mybir.AluOpType

MAGIC = float(1.5 * 2**23)

D = 1024
F = 4096
L = 4
NCORES = 8
TOK = 1024

NDK = D // 128    # 8 contraction tiles for up-proj
NFT = F // 128    # 32 contraction tiles for down-proj
NFC = F // 512    # 8 free-dim chunks for up-proj
NCH = F // 1024   # 4 column-chunks in the repacked wg/wu layout
NTT = TOK // 128  # 8 token tiles
PRE = 2           # q-pass lookahead depth (tiles)


def _split_excess_waits(nc, max_waits: int = 1) -> int:
    """walrus in this container rejects >1 sync-wait per instruction; split
    extras into standalone event-semaphore waits on the same engine."""
    n = 0
    for func in nc.m.functions:
        for block in func.blocks:
            changed = False
            out = []
            for inst in block.instructions:
                si = getattr(inst, "sync_info", None)
                if si is not None and si.on_wait and len(si.on_wait) > max_waits:
                    waits = list(si.on_wait)
                    for j, w in enumerate(waits[max_waits:]):
                        out.append(
                            mybir.InstEventSemaphore(
                                name=f"{inst.name}-xw{j}",
                                engine=inst.engine,
                                ins=[],
                                outs=[],
                                sync_info=mybir.SyncInfo(on_wait=[w], on_update=[]),
                            )
                        )
                        n += 1
                    inst.sync_info = mybir.SyncInfo(
                        on_wait=waits[:max_waits], on_update=list(si.on_update)
                    )
                    changed = True
                out.append(inst)
            if changed:
                block.instructions = out
    return n


def build(n_cores: int = NCORES, n_tok_tiles: int = NTT, n_layers: int = L) -> bass.Bass:
    nc = bass.Bass(num_devices=n_cores)
    tok = n_tok_tiles * 128
    ntt = n_tok_tiles

    x_ext = nc.declare_dram_parameter("x", [tok, D], dt.float32, isOutput=False)
    rs_ext = nc.declare_dram_parameter("rs", [n_layers, D], dt.float32, isOutput=False)
    # wg/wu repacked host-side to [L, NCH, NDK, 128, 1024]: every (ch, dk)
    # weight tile is one contiguous 512 KB stream unit.
    wg_ext = nc.declare_dram_parameter("wg", [n_layers, NCH, NDK, 128, 1024], dt.float32, isOutput=False)
    wu_ext = nc.declare_dram_parameter("wu", [n_layers, NCH, NDK, 128, 1024], dt.float32, isOutput=False)
    wd_ext = nc.declare_dram_parameter("wd", [n_layers, F, D], dt.float32, isOutput=False)
    # host-computed per-matrix quant scales 127/(max|W|+1e-5), order 3l+{g,u,d}
    wscin_ext = nc.declare_dram_parameter("wscin", [1, 16], dt.float32, isOutput=False)
    out_ext = nc.declare_dram_parameter("out", [tok, D], dt.float32, isOutput=True)

    with tile.TileContext(nc) as tc, ExitStack() as ctx:
        P = ctx.enter_context
        const = P(tc.tile_pool(name="const", bufs=1))
        wpool = P(tc.tile_pool(name="wts", bufs=3))
        xpool = P(tc.tile_pool(name="x1", bufs=4))
        t1pool = P(tc.tile_pool(name="t1", bufs=2))
        qpool = P(tc.tile_pool(name="q", bufs=1))
        qtpool = P(tc.tile_pool(name="qt", bufs=3))
        gpool = P(tc.tile_pool(name="g", bufs=2))
        gqpool = P(tc.tile_pool(name="gq", bufs=1))
        gqtpool = P(tc.tile_pool(name="gqt", bufs=1))
        silupool = P(tc.tile_pool(name="silu", bufs=1))
        xdrpool = P(tc.tile_pool(name="xdr", bufs=1))
        wst = P(tc.tile_pool(name="wst", bufs=3))
        wi16 = P(tc.tile_pool(name="wi16", bufs=1))
        w8s = P(tc.tile_pool(name="w8s", bufs=2))
        sbcpool = P(tc.tile_pool(name="sbc", bufs=2))
        rsrow = P(tc.tile_pool(name="rsrow", bufs=1))
        batch = P(tc.tile_pool(name="batch", bufs=2))
        sc = P(tc.tile_pool(name="sc", bufs=4))
        dram = P(tc.tile_pool(name="dram", bufs=2, space="DRAM"))
        psA = P(tc.tile_pool(name="psA", bufs=2, space="PSUM"))
        psB = P(tc.tile_pool(name="psB", bufs=3, space="PSUM"))
        psD = P(tc.tile_pool(name="psD", bufs=2, space="PSUM"))
        psM = P(tc.tile_pool(name="psM", bufs=1, space="PSUM"))

        # ---------- constants ----------
        ones1 = const.tile([1, 128], dt.float32, tag="ones1")
        nc.gpsimd.memset(ones1[:], 1.0)
        ones1b = const.tile([1, 128], dt.bfloat16, tag="ones1b")
        nc.gpsimd.memset(ones1b[:], 1.0)
        mag = const.tile([128, 1], dt.float32, tag="mag")
        nc.gpsimd.memset(mag[:], MAGIC)
        nmag = const.tile([128, 1], dt.float32, tag="nmag")
        nc.gpsimd.memset(nmag[:], -MAGIC)
        wsc = const.tile([128, 16], dt.float32, tag="wsc")
        grow = const.tile([1, 16], dt.float32, tag="grow")
        nc.sync.dma_start(grow[:], wscin_ext[:, :])
        wsc_ps = psM.tile([128, 512], dt.float32, tag="psM", name="wscps")
        nc.tensor.matmul(wsc_ps[:, 0:16], ones1[:], grow[:], start=True, stop=True)
        nc.scalar.activation(wsc[:], wsc_ps[:, 0:16], AF.Copy)

        # ---------- rms_scale broadcast (bf16) ----------
        sbc = {}

        def bcast_scale(l):
            row = rsrow.tile([1, D], dt.bfloat16, tag="rsrow")
            nc.gpsimd.dma_start(row[:], rs_ext[l:l + 1, :])
            t = sbcpool.tile([128, D], dt.bfloat16, tag="sbc")
            for h in range(D // 512):
                ps = psM.tile([128, 512], dt.float32, tag="psM", name="bcps")
                nc.tensor.matmul(ps[:], ones1b[:], row[:, h * 512:(h + 1) * 512],
                                 start=True, stop=True)
                nc.scalar.activation(t[:, h * 512:(h + 1) * 512], ps[:], AF.Copy)
            sbc[l] = t

        bcast_scale(0)
        if n_layers > 1:
            bcast_scale(1)

        # ---------- phase A for layer 0 (transient x reads, row stats) ----------
        ssq = {0: batch.tile([128, ntt], dt.float32, tag="ssq", name="ssq0")}
        mxs = {0: batch.tile([128, ntt], dt.float32, tag="mx", name="mx0")}
        for i in range(ntt):
            xa = t1pool.tile([128, D], dt.float32, tag="t1")
            nc.sync.dma_start(xa[:], x_ext[i * 128:(i + 1) * 128, :])
            tb = t1pool.tile([128, D], dt.float32, tag="t1")
            nc.scalar.activation(tb[:], xa[:], AF.Square,
                                 accum_out=ssq[0][:, i:i + 1])
            nc.vector.tensor_tensor(tb[:], xa[:], sbc[0][:], op=ALU.mult)
            nc.vector.tensor_reduce(mxs[0][:, i:i + 1], tb[:],
                                    axis=mybir.AxisListType.X, op=ALU.max,
                                    apply_absolute_value=True)

        # ---------- batched row stats -> c1 (=s*rstd), rs (=1/s) ----------
        c1_all, rs_all, stats_t = {}, {}, {}

        def stats(l, lo, hi):
            if lo == 0:
                ms = batch.tile([128, ntt], dt.float32, tag="ms")
                rt = batch.tile([128, ntt], dt.float32, tag="rt")
                rstd = batch.tile([128, ntt], dt.float32, tag="rstd")
                nwt = batch.tile([128, ntt], dt.float32, tag="nwt")
                maxh = batch.tile([128, ntt], dt.float32, tag="maxh")
                sr = batch.tile([128, ntt], dt.float32, tag="sr")
                s_all = batch.tile([128, ntt], dt.float32, tag="s_all")
                c1 = batch.tile([128, ntt], dt.float32, tag="c1")
                rs = batch.tile([128, ntt], dt.float32, tag="rs_all")
                stats_t[l] = (ms, rt, rstd, nwt, maxh, sr, s_all, c1, rs)
                c1_all[l], rs_all[l] = c1, rs
            ms, rt, rstd, nwt, maxh, sr, s_all, c1, rs = stats_t[l]
            s_ = slice(lo, hi)
            nc.vector.tensor_scalar(ms[:, s_], ssq[l][:, s_], 1.0 / D, 1e-6, op0=ALU.mult, op1=ALU.add)
            nc.scalar.activation(rt[:, s_], ms[:, s_], AF.Sqrt)
            nc.vector.reciprocal(rstd[:, s_], rt[:, s_])
            # one Newton step fixes the Sqrt-LUT error that quantization
            # tie-flips amplify layer by layer
            nc.vector.tensor_tensor(nwt[:, s_], rstd[:, s_], rstd[:, s_], op=ALU.mult)
            nc.vector.tensor_tensor(nwt[:, s_], nwt[:, s_], ms[:, s_], op=ALU.mult)
            nc.vector.tensor_scalar(nwt[:, s_], nwt[:, s_], -0.5, 1.5, op0=ALU.mult, op1=ALU.add)
            nc.vector.tensor_tensor(rstd[:, s_], rstd[:, s_], nwt[:, s_], op=ALU.mult)
            nc.vector.tensor_tensor(maxh[:, s_], mxs[l][:, s_], rstd[:, s_], op=ALU.mult)
            nc.vector.tensor_scalar(maxh[:, s_], maxh[:, s_], 1e-5, None, op0=ALU.add)
            nc.vector.reciprocal(sr[:, s_], maxh[:, s_])
            nc.vector.tensor_scalar(s_all[:, s_], sr[:, s_], 127.0, 1e3, op0=ALU.mult, op1=ALU.min)
            nc.vector.tensor_scalar(s_all[:, s_], s_all[:, s_], 1e-3, None, op0=ALU.max)
            nc.vector.tensor_tensor(c1[:, s_], s_all[:, s_], rstd[:, s_], op=ALU.mult)
            nc.vector.reciprocal(rs[:, s_], s_all[:, s_])

        stats(0, 0, ntt)

        # ---------- q-pass ----------
        xs, qts = {}, {}

        def q_pass(l, i, xsrc):
            x1 = xpool.tile([128, D], dt.float32, tag="x1")
            nc.sync.dma_start(x1[:], xsrc[i * 128:(i + 1) * 128, :])
            xs[(l, i)] = x1
            t1 = t1pool.tile([128, D], dt.float32, tag="t1")
            nc.vector.tensor_tensor(t1[:], x1[:], sbc[l][:], op=ALU.mult)
            nc.scalar.activation(t1[:], t1[:], AF.Identity,
                                 scale=c1_all[l][:, i:i + 1], bias=mag[:])
            q = qpool.tile([128, D], dt.bfloat16, tag="q")
            nc.scalar.activation(q[:], t1[:], AF.Identity, bias=nmag[:])
            qT = qtpool.tile([128, NDK, 128], dt.bfloat16, tag="qt")
            nc.sync.dma_start_transpose(qT[:], q[:])
            qts[(l, i)] = qT

        q_pass(0, 0, x_ext)
        q_pass(0, 1, x_ext)

        # ---------- ternarize ([128, 1024] units) ----------
        pending_wr = []

        def flush_wr(keep=0):
            while len(pending_wr) > keep:
                dst, s8 = pending_wr.pop(0)
                nc.gpsimd.dma_start(dst, s8[:])

        def tern_unit(src_ap, idx, dst_sb=None, dst_dram=None):
            wt = wst.tile([128, 1024], dt.float32, tag="wst")
            nc.sync.dma_start(wt[:], src_ap)
            r = wi16.tile([128, 1024], dt.int16, tag="wi16")
            nc.vector.tensor_scalar(r[:], wt[:], wsc[:, idx:idx + 1], None, op0=ALU.mult)
            if dst_sb is not None:
                nc.vector.tensor_scalar(dst_sb, r[:], 1, -1, op0=ALU.min, op1=ALU.max)
            else:
                s8 = w8s.tile([128, 1024], dt.float8e4, tag="w8s")
                nc.vector.tensor_scalar(s8[:], r[:], 1, -1, op0=ALU.min, op1=ALU.max)
                pending_wr.append((dst_dram, s8))

        def tern_jobs(l, g8dst, u8dst, d8dst):
            # order: ch0 wg/wu, ch1, wd, ch2, ch3 so the first up-proj
            # f-chunks and the first down-proj unblock earliest at layer 0
            def wgu(ch):
                out = []
                for dk in range(NDK):
                    out.append((wg_ext[l, ch, dk], 3 * l,
                                g8dst[:, dk, ch * 1024:(ch + 1) * 1024]))
                    out.append((wu_ext[l, ch, dk], 3 * l + 1,
                                u8dst[:, dk, ch * 1024:(ch + 1) * 1024]))
                return out
            jobs = wgu(0) + wgu(1)
            for ft in range(NFT):
                jobs.append((wd_ext[l, ft * 128:(ft + 1) * 128, :], 3 * l + 2,
                             d8dst[:, ft, :]))
            jobs += wgu(2) + wgu(3)
            return jobs

        # layer-0: ternarize straight into the resident fp8 tiles
        wg_t = wpool.tile([128, NDK, F], dt.float8e4, tag="wts")
        wu_t = wpool.tile([128, NDK, F], dt.float8e4, tag="wts")
        wd_t = wpool.tile([128, NFT, D], dt.float8e4, tag="wts")
        for src_ap, idx, dst in tern_jobs(0, wg_t, wu_t, wd_t):
            tern_unit(src_ap, idx, dst_sb=dst)

        # ---------- main layer loop ----------
        xbuf = {}
        stage8 = {}
        for l in range(n_layers):
            if l > 0:
                g8, u8, d8 = stage8[l]
                wg_t = wpool.tile([128, NDK, F], dt.float8e4, tag="wts")
                wu_t = wpool.tile([128, NDK, F], dt.float8e4, tag="wts")
                wd_t = wpool.tile([128, NFT, D], dt.float8e4, tag="wts")
                nc.gpsimd.dma_start(wg_t[:], g8[:])
                nc.gpsimd.dma_start(wu_t[:], u8[:])
                nc.gpsimd.dma_start(wd_t[:], d8[:])
                xsrc = xbuf[l - 1]
            else:
                xsrc = x_ext

            if l + 1 < n_layers:
                ssq[l + 1] = batch.tile([128, ntt], dt.float32, tag="ssq", name="ssqn")
                mxs[l + 1] = batch.tile([128, ntt], dt.float32, tag="mx", name="mxn")
                if l + 1 > 1:
                    bcast_scale(l + 1)
                g8 = dram.tile([128, NDK, F], dt.float8e4, tag="wg8")
                u8 = dram.tile([128, NDK, F], dt.float8e4, tag="wu8")
                d8 = dram.tile([128, NFT, D], dt.float8e4, tag="wd8")
                stage8[l + 1] = (g8, u8, d8)
                jobs = tern_jobs(l + 1, g8, u8, d8)
            else:
                jobs = []

            if l == n_layers - 1:
                xdst = out_ext
            else:
                xdst = dram.tile([tok, D], dt.float32, tag="xbuf")
                xbuf[l] = xdst

            # tern jobs spread over tile slots 2..ntt-1, interleaved per fg
            nslots = ntt - 2
            jper = (len(jobs) + nslots - 1) // nslots if jobs else 0

            pend = {}       # (l, i) -> (x1, stile, gqT)    until down-proj drains

            def emit_down(l_, i_):
                x1, stile, gqT = pend.pop((l_, i_))
                xd0 = psD.tile([128, 512], dt.float32, tag="xdps")
                xd1 = psD.tile([128, 512], dt.float32, tag="xdps")
                for ft in range(NFT):
                    nc.tensor.matmul(xd0[:], gqT[:, ft, :], wd_t[:, ft, 0:512],
                                     start=(ft == 0), stop=(ft == NFT - 1))
                    nc.tensor.matmul(xd1[:], gqT[:, ft, :], wd_t[:, ft, 512:1024],
                                     start=(ft == 0), stop=(ft == NFT - 1))
                for dc, xd_ps in ((0, xd0), (1, xd1)):
                    xdr = xdrpool.tile([128, 512], dt.float32, tag="xdr")
                    nc.scalar.activation(xdr[:], xd_ps[:], AF.Copy,
                                         scale=stile[:, 15:16])
                    nc.vector.tensor_tensor(
                        x1[:, dc * 512:(dc + 1) * 512],
                        x1[:, dc * 512:(dc + 1) * 512], xdr[:], op=ALU.add)
                nc.sync.dma_start(xdst[i_ * 128:(i_ + 1) * 128, :], x1[:])
                if l_ + 1 < n_layers:
                    t1 = t1pool.tile([128, D], dt.float32, tag="t1")
                    nc.scalar.activation(t1[:], x1[:], AF.Square,
                                         accum_out=ssq[l_ + 1][:, i_:i_ + 1])
                    nc.vector.tensor_tensor(t1[:], x1[:], sbc[l_ + 1][:], op=ALU.mult)
                    nc.vector.tensor_reduce(mxs[l_ + 1][:, i_:i_ + 1], t1[:],
                                            axis=mybir.AxisListType.X, op=ALU.max,
                                            apply_absolute_value=True)

            for i in range(ntt):
                x1 = xs.pop((l, i))
                qT = qts.pop((l, i))

                # ---- up-projection + GLU ----
                g = gpool.tile([128, F], dt.float32, tag="g")
                stile = sc.tile([128, 16], dt.float32, tag="stile")
                slot_jobs = jobs[(i - 2) * jper:(i - 1) * jper] if i >= 2 else []
                for fg in range(NFC):
                    u_ps = psA.tile([128, 512], dt.float32, tag="ups")
                    v_ps = psB.tile([128, 512], dt.float32, tag="vps")
                    for dk in range(NDK):
                        nc.tensor.matmul(
                            u_ps[:], qT[:, dk, :], wg_t[:, dk, fg * 512:(fg + 1) * 512],
                            start=(dk == 0), stop=(dk == NDK - 1))
                        nc.tensor.matmul(
                            v_ps[:], qT[:, dk, :], wu_t[:, dk, fg * 512:(fg + 1) * 512],
                            start=(dk == 0), stop=(dk == NDK - 1))
                    su = silupool.tile([128, 512], dt.float32, tag="silu")
                    nc.scalar.activation(su[:], u_ps[:], AF.Silu,
                                         scale=rs_all[l][:, i:i + 1])
                    nc.vector.tensor_tensor(
                        g[:, fg * 512:(fg + 1) * 512], su[:], v_ps[:], op=ALU.mult)
                    nc.vector.tensor_reduce(
                        stile[:, fg:fg + 1], g[:, fg * 512:(fg + 1) * 512],
                        axis=mybir.AxisListType.X, op=ALU.max,
                        apply_absolute_value=True)

                # ---- s2 = clip(127/(max|g|/s + 1e-5)); c2 = s2/s; rs2 = 1/s2 ----
                nc.vector.tensor_reduce(
                    stile[:, 8:9], stile[:, 0:8], axis=mybir.AxisListType.X,
                    op=ALU.max, apply_absolute_value=False)
                nc.vector.tensor_tensor(stile[:, 9:10], stile[:, 8:9],
                                        rs_all[l][:, i:i + 1], op=ALU.mult)
                nc.vector.tensor_scalar(stile[:, 10:11], stile[:, 9:10], 1e-5, None,
                                        op0=ALU.add)
                nc.vector.reciprocal(stile[:, 11:12], stile[:, 10:11])
                nc.vector.tensor_scalar(stile[:, 12:13], stile[:, 11:12], 127.0, 1e3,
                                        op0=ALU.mult, op1=ALU.min)
                nc.vector.tensor_scalar(stile[:, 13:14], stile[:, 12:13], 1e-3, None,
                                        op0=ALU.max)
                nc.vector.tensor_tensor(stile[:, 14:15], stile[:, 13:14],
                                        rs_all[l][:, i:i + 1], op=ALU.mult)
                nc.vector.reciprocal(stile[:, 15:16], stile[:, 13:14])

                # ---- gq = round(c2*g) via magic ----
                nc.scalar.activation(g[:], g[:], AF.Identity,
                                     scale=stile[:, 14:15], bias=mag[:])
                gq = gqpool.tile([128, F], dt.bfloat16, tag="gq")
                nc.scalar.activation(gq[:], g[:], AF.Identity, bias=nmag[:])

                # ---- previous tile's down-projection runs under this chain ----
                if i > 0:
                    emit_down(l, i - 1)

                gqT = gqtpool.tile([128, NFT, 128], dt.bfloat16, tag="gqt")
                nc.sync.dma_start_transpose(gqT[:], gq[:])
                pend[(l, i)] = (x1, stile, gqT)

                if i + PRE < ntt:
                    q_pass(l, i + PRE, xsrc)
                for src_ap, idx, dst in slot_jobs:
                    flush_wr(keep=1)
                    tern_unit(src_ap, idx, dst_dram=dst)
                if i == ntt - 1:
                    flush_wr(keep=0)
                if l + 1 < n_layers:
                    if i == ntt // 2:
                        stats(l + 1, 0, ntt // 2)
                    if i == ntt - 2:
                        q_pass(l + 1, 0, xdst)
                    if i == ntt - 1:
                        q_pass(l + 1, 1, xdst)

            emit_down(l, ntt - 1)
            if l + 1 < n_layers:
                stats(l + 1, ntt // 2, ntt)

    _split_excess_waits(nc)
    return nc


_nc_cache = {}


def _get_nc(key=(NCORES, NTT, L)):
    if key not in _nc_cache:
        _nc_cache[key] = build(*key)
    return _nc_cache[key]


def _repack(w, n_layers):
    # [L, D, F] -> [L, F//1024, D//128, 128, 1024]: each (ch, dk) tile is one
    # contiguous 512 KB stream unit
    return np.ascontiguousarray(
        w.reshape(n_layers, D // 128, 128, F // 1024, 1024).transpose(0, 3, 1, 2, 4)
    )


def _make_in_maps(x, rs, wg, wu, wd, n_cores=NCORES):
    n_layers = rs.shape[0]
    wg_r = _repack(wg, n_layers)
    wu_r = _repack(wu, n_layers)
    wscin = np.zeros((1, 16), dtype=np.float32)
    for l in range(n_layers):
        for mi, w in enumerate((wg, wu, wd)):
            m = np.float32(np.abs(w[l]).max())
            wscin[0, 3 * l + mi] = np.float32(127.0) / (m + np.float32(1e-5))
    in_maps = []
    for c in range(n_cores):
        in_maps.append({
            "x": x[c],
            "rs": rs,
            "wg": wg_r,
            "wu": wu_r,
            "wd": wd,
            "wscin": wscin,
        })
    return in_maps


def kernel(x, rms_scale, W_g, W_u, W_d):
    """Full-input entry point: shard over batch, run 8-core SPMD, gather."""
    x = np.ascontiguousarray(np.asarray(x, dtype=np.float32))
    rs = np.ascontiguousarray(np.asarray(rms_scale, dtype=np.float32))
    wg = np.ascontiguousarray(np.asarray(W_g, dtype=np.float32))
    wu = np.ascontiguousarray(np.asarray(W_u, dtype=np.float32))
    wd = np.ascontiguousarray(np.asarray(W_d, dtype=np.float32))
    B, S, Dx = x.shape
    assert (B, S, Dx) == (NCORES, TOK, D), (B, S, Dx)
    nc = _get_nc()
    in_maps = _make_in_maps(x, rs, wg, wu, wd)
    res = run_bass_kernel_spmd(nc, in_maps, list(range(NCORES)))
    return np.stack([res.results[c]["out"] for c in range(NCORES)], axis=0)



# revision 25
# speedup vs baseline: 1.0915x; 1.0915x over previous
"""Trainium2 Bass kernel for nn_CascadeTransformerMM (4-layer ternary-GLU cascade).

v9: feature-major (transposed) dataflow.
  - Host ternarizes the weights exactly (sign(round(w*scale)) in fp32 RNE)
    and ships them as fp8e4 {-1,0,+1}; no on-device ternarization, no fp32
    weight streaming, no DRAM staging.
  - Activations live as X[d, t] (d on partitions), SBUF-resident across all
    4 layers.  Up-proj makes U,V as [f, t] (weights stationary, q moving);
    [f, t] feeds the down-proj directly, so the qT/gqT DMA-transpose storm
    of v8 (231K descriptors) is gone entirely.
  - Cross-partition stats (ssq, max|h|, max|g|) via gpsimd
    partition_all_reduce; per-token scales become broadcast [128, t] tiles
    consumed by DVE tensor_tensor; stat chains run redundantly on broadcast
    tiles (identical op sequence to v8's per-column chains).
  - 16 sweeps of 256 tokens, software-pipelined: phase-2 stat latency is
    hidden under 2 pre-issued up-proj chunks of the next sweep.

Math per layer (identical to v8):
  h = rms_scale * x * rstd;  s = clip(127/(max|h|+1e-5), 1e-3, 1e3)
  q = round(s*h)  (magic-number round, bf16-exact ints)
  U = q@Wg_t; V = q@Wu_t  (ternary fp8 weights, int-exact fp32 accum)
  g_b = silu(U*rs) * V;  s2 = clip(127/(max|g_b|*rs + 1e-5), ...)
  gq = round(g_b * s2*rs);  x += (gq@Wd_t) * (1/s2)

Distribution: data-parallel over batch (8 batches -> 8 cores).
"""

import os
import sys

for _p in ("/opt/trn_rl_repo", "/root/.axon_site/_ro/trn_rl_repo"):
    if os.path.isdir(_p) and _p not in sys.path:
        sys.path.insert(0, _p)

import numpy as np
import ml_dtypes
from contextlib import ExitStack

import concourse.bass as bass
import concourse.mybir as mybir
import concourse.tile as tile
from concourse.bass_isa import ReduceOp
from concourse import library_config, library_overlay
from concourse.bass_utils import run_bass_kernel_spmd

dt = mybir.dt
AF = mybir.ActivationFunctionType
ALU = mybir.AluOpType

MAGIC = float(1.5 * 2**23)

D = 1024
F = 4096
L = 4
NCORES = 8
TOK = 1024

TOKT = 256            # tokens per sweep
NSW = TOK // TOKT     # sweeps per layer
NDJ = D // 128        # 8 d-tiles
NFC = F // 128        # 32 f-chunks
NOCT = 4              # wd octets per quad phase
FP8 = dt.float8e4


def _split_excess_waits(nc, max_waits: int = 1) -> int:
    """walrus in this container rejects >1 sync-wait per instruction; split
    extras into standalone event-semaphore waits on the same engine."""
    n = 0
    for func in nc.m.functions:
        for block in func.blocks:
            changed = False
            out = []
            for inst in block.instructions:
                si = getattr(inst, "sync_info", None)
                if si is not None and si.on_wait and len(si.on_wait) > max_waits:
                    waits = list(si.on_wait)
                    for j, w in enumerate(waits[max_waits:]):
                        out.append(
                            mybir.InstEventSemaphore(
                                name=f"{inst.name}-xw{j}",
                                engine=inst.engine,
                                ins=[],
                                outs=[],
                                sync_info=mybir.SyncInfo(on_wait=[w], on_update=[]),
                            )
                        )
                        n += 1
                    inst.sync_info = mybir.SyncInfo(
                        on_wait=waits[:max_waits], on_update=list(si.on_update)
                    )
                    changed = True
                out.append(inst)
            if changed:
                block.instructions = out
    return n


DEBUG = False


def build(n_cores: int = NCORES) -> bass.Bass:
    nc = bass.Bass(num_devices=n_cores)

    x_ext = nc.declare_dram_parameter("x", [D, TOK], dt.float32, isOutput=False)
    rsc_ext = nc.declare_dram_parameter("rsc", [128, L * NDJ], dt.float32, isOutput=False)
    # chunk-grouped layouts: 4 f-chunks (wg/wu) or one quad-octet (wd) per
    # DMA, 4KB contiguous per partition
    wg_ext = nc.declare_dram_parameter("wg", [L, NFC // 4, 128, 4, NDJ, 128], FP8, isOutput=False)
    wu_ext = nc.declare_dram_parameter("wu", [L, NFC // 4, 128, 4, NDJ, 128], FP8, isOutput=False)
    wd_ext = nc.declare_dram_parameter("wd", [L, 2, NOCT, 128, 4, 8, 128], FP8, isOutput=False)
    out_ext = nc.declare_dram_parameter("out", [D, TOK], dt.float32, isOutput=True)
    if DEBUG:
        dbg_ext = nc.declare_dram_parameter("dbg", [16, 128, TOKT], dt.float32, isOutput=True)

    with tile.TileContext(nc) as tc, ExitStack() as ctx:
        P = ctx.enter_context
        const = P(tc.tile_pool(name="const", bufs=1))
        xpool = P(tc.tile_pool(name="X", bufs=1))
        qpool = P(tc.tile_pool(name="q", bufs=16))
        gbpool = P(tc.tile_pool(name="gb", bufs=40))
        gqpool = P(tc.tile_pool(name="gq", bufs=40))
        t0pool = P(tc.tile_pool(name="t0", bufs=4))
        trpool = P(tc.tile_pool(name="tr", bufs=6))
        stpool = P(tc.tile_pool(name="st", bufs=2))
        bcpool = P(tc.tile_pool(name="bc", bufs=3))
        smpool = P(tc.tile_pool(name="sm", bufs=2))
        wgupool = P(tc.tile_pool(name="wgu", bufs=3))
        wdpool = P(tc.tile_pool(name="wd", bufs=4))
        psUV = P(tc.tile_pool(name="psUV", bufs=3, space="PSUM"))
        psX = P(tc.tile_pool(name="psX", bufs=1, space="PSUM"))

        # ---------- constants ----------
        nc.gpsimd.load_library(library_config.attn)
        mag = const.tile([128, 1], dt.float32, tag="mag")
        nc.gpsimd.memset(mag[:], MAGIC)
        nmag = const.tile([128, 1], dt.float32, tag="nmag")
        nc.gpsimd.memset(nmag[:], -MAGIC)
        rscol = const.tile([128, L * NDJ], dt.float32, tag="rscol")
        nc.sync.dma_start(rscol[:], rsc_ext[:, :])

        def sbc_ap(l, dj):
            return rscol[:, l * NDJ + dj:l * NDJ + dj + 1]

        def tap(i, ap):
            if DEBUG:
                nc.sync.dma_start(dbg_ext[i], ap)

        # ---------- persistent X tiles ----------
        X = {}
        for dj in range(NDJ):
            for s in range(NSW):
                t = xpool.tile([128, TOKT], dt.float32, tag=f"x{dj}_{s}", name=f"x{dj}_{s}")
                X[(dj, s)] = t

        # down-proj accumulator: [128, 8, TOKT] fp32 = 4 PSUM banks
        xps = psX.tile([128, NDJ, TOKT], dt.float32, tag="xps", name="xps")

        st = {}  # per-(l, s) state

        def S(l, s):
            return st.setdefault((l, s), {})

        # ---------- x load ----------
        def emit_xload(s):
            for dj in range(NDJ):
                nc.sync.dma_start(
                    X[(dj, s)][:],
                    x_ext[dj * 128:(dj + 1) * 128, s * TOKT:(s + 1) * TOKT])

        # ---------- stats gather (ssq + max|sbc*x|) from X tiles ----------
        def emit_stats_dj(l, s, dj):
            """Accumulate S (sum over d of x^2) and M (max over d of |sbc*x|)
            for layer l's phase-1 stats of sweep s; call per dj in order."""
            d = S(l, s)
            sq = trpool.tile([128, TOKT], dt.float32, tag="tr", name="sq")
            if dj == 0:
                d["S"] = smpool.tile([128, TOKT], dt.float32, tag="S", name="Ssum")
                d["M"] = smpool.tile([128, TOKT], dt.float32, tag="M", name="Mmax")
            nc.scalar.activation(sq[:], X[(dj, s)][:], AF.Square)
            if dj == 0:
                nc.vector.tensor_scalar(d["S"][:], sq[:], 0.0, None, op0=ALU.add)
            else:
                nc.vector.tensor_tensor(d["S"][:], d["S"][:], sq[:], op=ALU.add)
            sx = trpool.tile([128, TOKT], dt.float32, tag="tr", name="sx")
            nc.scalar.activation(sx[:], X[(dj, s)][:], AF.Abs, scale=sbc_ap(l, dj))
            if dj == 0:
                nc.vector.tensor_scalar(d["M"][:], sx[:], 0.0, None, op0=ALU.max)
            else:
                nc.vector.tensor_tensor(d["M"][:], d["M"][:], sx[:], op=ALU.max)

        # ---------- phase 1: rms + act-quant scales ----------
        def emit_phase1(l, s):
            d = S(l, s)
            nc.gpsimd.partition_all_reduce(d["S"][:], d["S"][:], 128, ReduceOp.add)
            nc.gpsimd.partition_all_reduce(d["M"][:], d["M"][:], 128, ReduceOp.max)
            ms = stpool.tile([128, TOKT], dt.float32, tag="st1", name="ms")
            rt = stpool.tile([128, TOKT], dt.float32, tag="st2", name="rt")
            rstd = stpool.tile([128, TOKT], dt.float32, tag="st3", name="rstd")
            nwt = stpool.tile([128, TOKT], dt.float32, tag="st4", name="nwt")
            nc.vector.tensor_scalar(ms[:], d["S"][:], 1.0 / D, 1e-6, op0=ALU.mult, op1=ALU.add)
            nc.scalar.activation(rt[:], ms[:], AF.Sqrt)
            nc.vector.reciprocal(rstd[:], rt[:])
            # Newton step refines the Sqrt-LUT rsqrt (v8-proven)
            nc.vector.tensor_tensor(nwt[:], rstd[:], rstd[:], op=ALU.mult)
            nc.vector.tensor_tensor(nwt[:], nwt[:], ms[:], op=ALU.mult)
            nc.vector.tensor_scalar(nwt[:], nwt[:], -0.5, 1.5, op0=ALU.mult, op1=ALU.add)
            nc.vector.tensor_tensor(rstd[:], rstd[:], nwt[:], op=ALU.mult)
            maxh = stpool.tile([128, TOKT], dt.float32, tag="st5", name="maxh")
            sr = stpool.tile([128, TOKT], dt.float32, tag="st6", name="sr")
            nc.vector.tensor_tensor(maxh[:], d["M"][:], rstd[:], op=ALU.mult)
            nc.vector.tensor_scalar(maxh[:], maxh[:], 1e-5, None, op0=ALU.add)
            nc.vector.reciprocal(sr[:], maxh[:])
            nc.vector.tensor_scalar(sr[:], sr[:], 127.0, 1e3, op0=ALU.mult, op1=ALU.min)
            nc.vector.tensor_scalar(sr[:], sr[:], 1e-3, None, op0=ALU.max)
            c1 = bcpool.tile([128, TOKT], dt.float32, tag="c1", name="c1bc")
            rs = bcpool.tile([128, TOKT], dt.float32, tag="rs", name="rsbc")
            nc.vector.tensor_tensor(c1[:], sr[:], rstd[:], op=ALU.mult)
            nc.vector.reciprocal(rs[:], sr[:])
            d["c1"], d["rs"] = c1, rs
            if (l, s) == (0, 0):
                tap(0, d["S"][:]); tap(1, d["M"][:]); tap(2, c1[:]); tap(3, rs[:])

        # ---------- q pass ----------
        def emit_qpass(l, s):
            d = S(l, s)
            qs = []
            for dj in range(NDJ):
                t0 = t0pool.tile([128, TOKT], dt.float32, tag="qt", name="qt0")
                nc.vector.tensor_tensor(t0[:], X[(dj, s)][:], d["c1"][:], op=ALU.mult)
                if (l, s, dj) == (0, 0, 0):
                    tap(4, t0[:])
                nc.scalar.activation(t0[:], t0[:], AF.Identity,
                                     scale=sbc_ap(l, dj), bias=mag[:])
                if (l, s, dj) == (0, 0, 0):
                    tap(15, t0[:])
                q = qpool.tile([128, TOKT], dt.bfloat16, tag="q", name="q")
                nc.scalar.activation(q[:], t0[:], AF.Identity, bias=nmag[:])
                qs.append(q)
            d["q"] = qs

        # ---------- up-proj + GLU, one f-chunk ----------
        def emit_uv_chunk(l, s, c):
            d = S(l, s)
            cg, ci = divmod(c, 4)
            wgu = d.setdefault("wgu", {})
            if cg not in wgu:
                wgt4 = wgupool.tile([128, 4, NDJ, 128], FP8, tag="wg", name="wgt")
                wut4 = wgupool.tile([128, 4, NDJ, 128], FP8, tag="wu", name="wut")
                nc.sync.dma_start(wgt4[:], wg_ext[l, cg])
                nc.sync.dma_start(wut4[:], wu_ext[l, cg])
                wgu[cg] = (wgt4, wut4)
            wgt4, wut4 = wgu[cg]
            wgt = wgt4[:, ci]
            wut = wut4[:, ci]
            uv = psUV.tile([128, 2, TOKT], dt.float32, tag="uv", name="uvps")
            qs = d["q"]
            # U and V share one 2KB PSUM bank; start=True zeroes the WHOLE
            # bank (ZERO_REGION_SIZE=2048), so only U's first matmul sets it
            # and V's first write consumes the pending-zero with start=False.
            for dk in range(NDJ):
                nc.tensor.matmul(uv[:, 0, :], wgt[:, dk, :], qs[dk][:],
                                 start=(dk == 0), stop=(dk == NDJ - 1))
                nc.tensor.matmul(uv[:, 1, :], wut[:, dk, :], qs[dk][:],
                                 start=False, stop=(dk == NDJ - 1))
            t0 = t0pool.tile([128, TOKT], dt.float32, tag="t0", name="silut")
            nc.vector.tensor_tensor(t0[:], uv[:, 0, :], d["rs"][:], op=ALU.mult)
            if (l, s, c) == (0, 0, 0):
                tap(14, t0[:])
            nc.scalar.activation(t0[:], t0[:], AF.Silu)
            if (l, s, c) == (0, 0, 0):
                tap(5, t0[:])
            gb = gbpool.tile([128, TOKT], dt.float32, tag="gb", name="gb")
            nc.vector.tensor_tensor(gb[:], t0[:], uv[:, 1, :], op=ALU.mult)
            if (l, s, c) == (0, 0, 0):
                tap(6, gb[:])
            ga = t0pool.tile([128, TOKT], dt.float32, tag="ga", name="ga")
            nc.scalar.activation(ga[:], gb[:], AF.Abs)
            if c == 0:
                d["mx"] = smpool.tile([128, TOKT], dt.float32, tag="mx", name="mx")
                nc.vector.tensor_scalar(d["mx"][:], ga[:], 0.0, None, op0=ALU.max)
            else:
                nc.vector.tensor_tensor(d["mx"][:], d["mx"][:], ga[:], op=ALU.max)
            d.setdefault("gb", []).append(gb)

        # ---------- phase 2: g quant scales ----------
        def emit_phase2(l, s):
            d = S(l, s)
            nc.gpsimd.partition_all_reduce(d["mx"][:], d["mx"][:], 128, ReduceOp.max)
            g1 = stpool.tile([128, TOKT], dt.float32, tag="g1", name="g1")
            g2 = stpool.tile([128, TOKT], dt.float32, tag="g2", name="g2")
            nc.vector.tensor_tensor(g1[:], d["mx"][:], d["rs"][:], op=ALU.mult)
            nc.vector.tensor_scalar(g1[:], g1[:], 1e-5, None, op0=ALU.add)
            nc.vector.reciprocal(g2[:], g1[:])
            nc.vector.tensor_scalar(g2[:], g2[:], 127.0, 1e3, op0=ALU.mult, op1=ALU.min)
            nc.vector.tensor_scalar(g2[:], g2[:], 1e-3, None, op0=ALU.max)  # = s2
            c2p = bcpool.tile([128, TOKT], dt.float32, tag="c2", name="c2p")
            rs2p = bcpool.tile([128, TOKT], dt.float32, tag="rs2", name="rs2p")
            nc.vector.tensor_tensor(c2p[:], g2[:], d["rs"][:], op=ALU.mult)
            nc.vector.reciprocal(rs2p[:], g2[:])
            d["c2p"], d["rs2p"] = c2p, rs2p
            if (l, s) == (0, 0):
                tap(7, d["mx"][:]); tap(8, c2p[:]); tap(9, rs2p[:])

        # ---------- gq rounding ----------
        def emit_gq(l, s):
            d = S(l, s)
            gqs = []
            for c in range(NFC):
                gb = d["gb"][c]
                nc.vector.tensor_tensor(gb[:], gb[:], d["c2p"][:], op=ALU.mult)
                nc.scalar.activation(gb[:], gb[:], AF.Identity, bias=mag[:])
                gq = gqpool.tile([128, TOKT], dt.bfloat16, tag="gq", name="gq")
                nc.scalar.activation(gq[:], gb[:], AF.Identity, bias=nmag[:])
                gqs.append(gq)
            d["gq"] = gqs

        # ---------- down-proj + residual + next-layer stats ----------
        def emit_wd_oct(l, Q, oct):
            t = wdpool.tile([128, 4, 8, 128], FP8, tag="wd", name="wdt")
            nc.sync.dma_start(t[:], wd_ext[l, Q, oct])
            return {Q * 4 + dji: t[:, dji] for dji in range(4)}

        def emit_down(l, s, pre_oct0):
            d = S(l, s)
            gqs = d["gq"]
            for Q in range(2):
                for oct in range(NOCT):
                    if Q == 0 and oct == 0:
                        wdt = pre_oct0
                    else:
                        wdt = emit_wd_oct(l, Q, oct)
                    for k in range(8):
                        ft = oct * 8 + k
                        for dji in range(4):
                            dj = Q * 4 + dji
                            # adjacent dj pairs share a PSUM bank: even dj's
                            # first matmul zeroes the bank, odd dj rides it
                            nc.tensor.matmul(
                                xps[:, dj, :], wdt[dj][:, k, :], gqs[ft][:],
                                start=(ft == 0 and dj % 2 == 0), stop=(ft == NFC - 1))
                for dji in range(4):
                    dj = Q * 4 + dji
                    upd = trpool.tile([128, TOKT], dt.float32, tag="tr", name="upd")
                    nc.vector.tensor_tensor(upd[:], xps[:, dj, :], d["rs2p"][:], op=ALU.mult)
                    if (s, dj) == (0, 0) and l == 0:
                        tap(10, upd[:])
                    nc.vector.tensor_tensor(X[(dj, s)][:], X[(dj, s)][:], upd[:], op=ALU.add)
                    if (s, dj) == (0, 0) and l < 3:
                        tap(11 + l, X[(dj, s)][:])
                    if l + 1 < L:
                        emit_stats_dj(l + 1, s, dj)
                    else:
                        nc.sync.dma_start(
                            out_ext[dj * 128:(dj + 1) * 128, s * TOKT:(s + 1) * TOKT],
                            X[(dj, s)][:])

        # ---------- main ----------
        for s in range(NSW):
            emit_xload(s)
        for dj in range(NDJ):
            emit_stats_dj(0, 0, dj)
        emit_phase1(0, 0)
        emit_qpass(0, 0)

        for l in range(L):
            for s in range(NSW):
                first = (l == 0 and s == 0)
                nl, ns = (l, s + 1) if s + 1 < NSW else (l + 1, 0)
                for c in range(0 if first else 6, NFC):
                    emit_uv_chunk(l, s, c)
                    if c == 8 and l == 0 and s + 1 < NSW:
                        for dj in range(NDJ):
                            emit_stats_dj(0, s + 1, dj)
                    if c == 12 and l == 0 and s + 1 < NSW:
                        emit_phase1(0, s + 1)
                        emit_qpass(0, s + 1)
                    if c == 8 and not (l == 0 and s + 1 < NSW) and nl < L:
                        emit_qpass(nl, ns)
                emit_phase2(l, s)
                # hide phase-2 + gq latency under pre-issued chunks of next sweep
                if nl < L:
                    for cpre in range(6):
                        emit_uv_chunk(nl, ns, cpre)
                pre_oct0 = emit_wd_oct(l, 0, 0)
                emit_gq(l, s)
                emit_down(l, s, pre_oct0)
                if l + 1 < L:
                    emit_phase1(l + 1, s)
                # drop consumed per-sweep state
                st.pop((l, s), None)

    library_overlay.lower_extended_insts(nc)
    _split_excess_waits(nc)
    return nc


_nc_cache = {}


def _get_nc(key=(NCORES,)):
    if key not in _nc_cache:
        _nc_cache[key] = build(*key)
    return _nc_cache[key]


def _ternarize(w):
    """Exact host-side ternarize: sign(round(w * 127/(max|w|+1e-5))) in
    fp32 RNE, matching ternarize(weight_quant(w)) in the reference."""
    w = np.ascontiguousarray(w, dtype=np.float32)
    m = np.float32(np.abs(w).max())
    s = np.float32(127.0) / (m + np.float32(1e-5))
    t = np.round(w * s)
    return np.clip(t, np.float32(-1.0), np.float32(1.0))


def _pack_weights(wg, wu, wd):
    f8 = ml_dtypes.float8_e4m3
    wg_p = np.empty((L, NFC // 4, 128, 4, NDJ, 128), dtype=f8)
    wu_p = np.empty((L, NFC // 4, 128, 4, NDJ, 128), dtype=f8)
    wd_p = np.empty((L, 2, NOCT, 128, 4, 8, 128), dtype=f8)
    for l in range(L):
        tg = _ternarize(wg[l])   # [D, F]
        tu = _ternarize(wu[l])
        td = _ternarize(wd[l])   # [F, D]
        # [dk, p, cg, ci, m] -> [cg, p, ci, dk, m]
        wg_p[l] = tg.reshape(NDJ, 128, NFC // 4, 4, 128).transpose(2, 1, 3, 0, 4).astype(f8)
        wu_p[l] = tu.reshape(NDJ, 128, NFC // 4, 4, 128).transpose(2, 1, 3, 0, 4).astype(f8)
        # [oct, k, p, Q, dji, m] -> [Q, oct, p, dji, k, m]
        wd_p[l] = td.reshape(NOCT, 8, 128, 2, 4, 128).transpose(3, 0, 2, 4, 1, 5).astype(f8)
    return np.ascontiguousarray(wg_p), np.ascontiguousarray(wu_p), np.ascontiguousarray(wd_p)


def _make_in_maps(x, rs, wg, wu, wd, n_cores=NCORES):
    wg_p, wu_p, wd_p = _pack_weights(wg, wu, wd)
    # rscol[p, l*8+dk] = rms_scale[l, dk*128+p]
    rsc = np.ascontiguousarray(
        rs.reshape(L, NDJ, 128).transpose(2, 0, 1).reshape(128, L * NDJ),
        dtype=np.float32)
    in_maps = []
    for c in range(n_cores):
        in_maps.append({
            "x": np.ascontiguousarray(x[c].T),   # [D, TOK]
            "rsc": rsc,
            "wg": wg_p,
            "wu": wu_p,
            "wd": wd_p,
        })
    return in_maps


def kernel(x, rms_scale, W_g, W_u, W_d):
    """Full-input entry point: shard over batch, run 8-core SPMD, gather."""
    x = np.ascontiguousarray(np.asarray(x, dtype=np.float32))
    rs = np.ascontiguousarray(np.asarray(rms_scale, dtype=np.float32))
    wg = np.ascontiguousarray(np.asarray(W_g, dtype=np.float32))
    wu = np.ascontiguousarray(np.asarray(W_u, dtype=np.float32))
    wd = np.ascontiguousarray(np.asarray(W_d, dtype=np.float32))
    B, Sx, Dx = x.shape
    assert (B, Sx, Dx) == (NCORES, TOK, D), (B, Sx, Dx)
    nc = _get_nc()
    in_maps = _make_in_maps(x, rs, wg, wu, wd)
    res = run_bass_kernel_spmd(nc, in_maps, list(range(NCORES)))
    return np.stack([np.ascontiguousarray(res.results[c]["out"].T)
                     for c in range(NCORES)], axis=0)
